# revision 18
# baseline (speedup 1.0000x reference)
"""Trainium2 Bass kernel for nn_MeshEncoder (GCN message passing, 8 NeuronCores).

Strategy (graph/data parallel, dst-sharded):
  - Nodes are sharded across 8 cores by destination (12500 real + 44 pad rows
    per core -> 98 tiles of 128).  Edges live on the core that owns their dst,
    sorted by (dst tile, src window), so segment_sum becomes a local PSUM
    accumulation: per 128-edge chunk we gather the source rows with
    nc.gpsimd.dma_gather and multiply with an on-device one-hot selection
    matrix (is_equal against an iota row) on the tensor engine.
  - Per layer each core builds a "message table": rows = (x * inv_deg) @ W_side
    for its node shard (the linear layer and the degree scaling commute with
    segment_sum, so only side_len features travel per edge).  Tables are
    exchanged with one AllGather per layer; edge gathers read the full table.
  - Final GCNMax layer aggregates raw x rows (aggregate-before-matmul), applies
    W_red afterwards, and reduces with a per-core running max + AllReduce(max).
All floating point math runs on device; the host only reorders/shards inputs
and derives integer index metadata (degree counts, sort orders, pad layout).
"""

import math
import os
import sys

for _p in ("/opt/trn_rl_repo",):
    if _p not in sys.path:
        sys.path.insert(0, _p)

import numpy as np

# ---------------------------------------------------------------------------
# Problem constants (hardcoded: harness runs kernel.py standalone)
# ---------------------------------------------------------------------------
N = 100_000
E = 1_600_000
LATENT = 512
ZERON_DIMS = [(3, 128), (128, 128), (128, 128), (128, 128), (128, 128),
              (128, 128), (128, 128), (128, 150), (150, 300)]
RED_DIM = (300, LATENT)
NCORES = 8
P = 128


def _side(fout):
    return max(fout // 3, 2)


def _pad64(x):
    return ((x + 63) // 64) * 64


# ---------------------------------------------------------------------------
# Host-side graph preprocessing (pure index manipulation)
# ---------------------------------------------------------------------------

def _graph_plan(edge_src, edge_dst, n_nodes, n_cores, nsh_real, n_windows):
    """Shard edges by dst core; per (core, dst-tile, src-window) build padded
    gather-index buckets with a globally uniform chunk count per window."""
    nsh = ((nsh_real + P - 1) // P) * P          # padded shard rows
    nt = nsh // P                                 # tiles per core
    padn = nsh * n_cores
    wrows = padn // n_windows                     # window rows (must be <=32768)
    assert padn % n_windows == 0 and wrows <= 32768

    src = np.asarray(edge_src, dtype=np.int64)
    dst = np.asarray(edge_dst, dtype=np.int64)
    # global padded id of a node
    gid = (src // nsh_real) * nsh + (src % nsh_real)
    core = dst // nsh_real
    dloc = dst % nsh_real                         # local dst within shard
    tile = dloc // P
    dl = dloc % P                                 # dst slot within tile
    win = gid // wrows
    sloc = gid % wrows                             # src row within window

    # bucket sizes [core, tile, window]
    counts = np.zeros((n_cores, nt, n_windows), dtype=np.int64)
    np.add.at(counts, (core, tile, win), 1)
    cw = np.maximum(1, ((counts.max(axis=(0, 1)) + P - 1) // P)).astype(int)
    ctot = int(cw.sum())

    # sort edges by (core, tile, window, sloc)
    order = np.lexsort((sloc, win, tile, core))
    core_s, tile_s, win_s, sloc_s, dl_s = (
        core[order], tile[order], win[order], sloc[order], dl[order])

    idx_arrays = []     # per core: list per window of int16 [16, L/16]
    dstloc_arrays = []  # per core: fp32 [128, nt*ctot]
    woff = np.concatenate([[0], np.cumsum(cw)])
    for c in range(n_cores):
        idx_w = []
        dloc_full = np.full((nt * ctot * P,), -1.0, dtype=np.float32)
        mask_c = core_s == c
        t_c, w_c, s_c, d_c = tile_s[mask_c], win_s[mask_c], sloc_s[mask_c], dl_s[mask_c]
        for w in range(n_windows):
            L = nt * cw[w] * P
            flat = np.zeros((L,), dtype=np.int16)   # pad with row 0 (safe)
            m = w_c == w
            t_b, s_b, d_b = t_c[m], s_c[m], d_c[m]
            # position within tile bucket
            # edges already sorted by tile; compute offsets per tile
            cnt = np.bincount(t_b, minlength=nt)
            starts = np.concatenate([[0], np.cumsum(cnt)])[:-1]
            pos_in_tile = np.arange(t_b.size) - starts[t_b]
            assert (pos_in_tile < cw[w] * P).all()
            slot = t_b * (cw[w] * P) + pos_in_tile
            flat[slot] = s_b.astype(np.int16)
            wrapped = flat.reshape(L // 16, 16).T     # [16, L/16]
            idx_w.append(np.ascontiguousarray(
                np.tile(wrapped, (8, 1))))            # [128, L/16] (per-Q7 copy)
            # dstloc: chunk col = t*ctot + woff[w] + k ; slot-in-chunk = partition
            k = pos_in_tile // P
            p_slot = pos_in_tile % P
            col = t_b * ctot + woff[w] + k
            dloc_full[col * P + p_slot] = d_b.astype(np.float32)
        idx_arrays.append(idx_w)
        dstloc_arrays.append(np.ascontiguousarray(
            dloc_full.reshape(nt * ctot, P).T))      # [128, nt*ctot]

    # inv degree (index-derived metadata), per core [128, nt]
    deg = np.bincount(dst, minlength=n_nodes).astype(np.float32)
    inv = 1.0 / np.maximum(deg, 1.0)
    inv_pad = np.zeros((n_cores * nsh,), dtype=np.float32)
    for c in range(n_cores):
        inv_pad[c * nsh: c * nsh + nsh_real] = inv[c * nsh_real:(c + 1) * nsh_real]
    invdeg_arrays = [
        np.ascontiguousarray(inv_pad[c * nsh:(c + 1) * nsh].reshape(nt, P).T)
        for c in range(n_cores)]

    return dict(nsh=nsh, nt=nt, padn=padn, wrows=wrows, cw=[int(x) for x in cw],
                ctot=ctot, woff=[int(x) for x in woff],
                idx=idx_arrays, dstloc=dstloc_arrays, invdeg=invdeg_arrays,
                nsh_real=nsh_real)


# ---------------------------------------------------------------------------
# Device program
# ---------------------------------------------------------------------------

def _build_program(plan, dims, red_dim, tg, tg_red):
    import concourse.bacc as bacc
    import concourse.mybir as mybir
    import concourse.tile as tile
    from concourse.masks import make_identity

    f32 = mybir.dt.float32
    i16 = mybir.dt.int16
    i32 = mybir.dt.int32
    Alu = mybir.AluOpType
    Act = mybir.ActivationFunctionType

    nsh, nt, padn, wrows = plan["nsh"], plan["nt"], plan["padn"], plan["wrows"]
    cw, ctot, woff = plan["cw"], plan["ctot"], plan["woff"]
    nw = len(cw)
    nlay = len(dims)
    sides = [_side(fo) for _, fo in dims]
    rws = [_pad64(s) for s in sides]
    f_red_in, f_red_out = red_dim
    rw_red = _pad64(f_red_in)
    rg = [list(range(NCORES))]

    nc = bacc.Bacc("TRN2", target_bir_lowering=False, debug=False,
                   enable_asserts=False, num_devices=NCORES)

    # ---- I/O ----
    posT = nc.dram_tensor("posT", [dims[0][0], nsh], f32, kind="ExternalInput").ap()
    w_in, b_in = [], []
    for l, (fi, fo) in enumerate(dims):
        w_in.append(nc.dram_tensor(f"W{l}", [fi, fo], f32, kind="ExternalInput").ap())
        b_in.append(nc.dram_tensor(f"b{l}", [fo, 1], f32, kind="ExternalInput").ap())
    wred_in = nc.dram_tensor("Wred", [f_red_in, f_red_out], f32, kind="ExternalInput").ap()
    bred_in = nc.dram_tensor("bred", [f_red_out, 1], f32, kind="ExternalInput").ap()
    idx_in = [nc.dram_tensor(f"idx{w}", [P, nt * cw[w] * 8], i16,
                             kind="ExternalInput").ap() for w in range(nw)]
    dstloc_in = nc.dram_tensor("dstloc", [P, nt * ctot], f32, kind="ExternalInput").ap()
    invdeg_in = nc.dram_tensor("invdeg", [P, nt], f32, kind="ExternalInput").ap()
    out_ext = nc.dram_tensor("out", [P, f_red_out // P], f32, kind="ExternalOutput").ap()

    # ---- internal DRAM ----
    xt_dram = [posT]
    for l in range(1, nlay):
        xt_dram.append(nc.dram_tensor(f"xT{l}", [dims[l][0], nsh], f32).ap())
    xt_dram.append(nc.dram_tensor(f"xT{nlay}", [f_red_in, nsh], f32).ap())
    uniq_rw = sorted(set(rws + [rw_red]))
    tbl_shard = {rw: nc.dram_tensor(f"tsh{rw}", [nsh, rw], f32).ap() for rw in uniq_rw}
    tbl_full = {rw: nc.dram_tensor(f"tfl{rw}", [padn, rw], f32,
                                   addr_space="Shared").ap() for rw in uniq_rw}
    maxsh = nc.dram_tensor("maxsh", [P, f_red_out // P], f32).ap()
    maxfull = nc.dram_tensor("maxfull", [P, f_red_out // P], f32,
                             addr_space="Shared").ap()

    def kch(fi):
        return [(k, min(k + P, fi)) for k in range(0, fi, P)]

    with tile.TileContext(nc) as tc:
        import contextlib
        with contextlib.ExitStack() as ctx:
            res = ctx.enter_context(tc.tile_pool(name="res", bufs=1))
            # resident SBUF: weights chunked along K (partition dim <= 128),
            # biases chunked to the exact row-ranges the combine phase uses.
            wsb = []          # wsb[l][ki] : [k1-k0, fo]
            bsb_agg = []      # [s, 1]
            bsb_pass = []     # [ (p1-p0, 1) per pass chunk ]
            for l, (fi, fo) in enumerate(dims):
                s = sides[l]
                chunks = []
                for ki, (k0, k1) in enumerate(kch(fi)):
                    t = res.tile([k1 - k0, fo], f32, tag=f"W{l}_{ki}")
                    nc.sync.dma_start(t[:], w_in[l][k0:k1, :])
                    chunks.append(t)
                wsb.append(chunks)
                tb = res.tile([s, 1], f32, tag=f"bA{l}")
                nc.sync.dma_start(tb[:], b_in[l][0:s, :])
                bsb_agg.append(tb)
                pieces = []
                for pi, (p0, p1) in enumerate(kch(fo - s)):
                    tp = res.tile([p1 - p0, 1], f32, tag=f"bP{l}_{pi}")
                    nc.sync.dma_start(tp[:], b_in[l][s + p0:s + p1, :])
                    pieces.append(tp)
                bsb_pass.append(pieces)
            wredsb = []
            for ki, (k0, k1) in enumerate(kch(f_red_in)):
                t = res.tile([k1 - k0, f_red_out], f32, tag=f"Wred_{ki}")
                nc.sync.dma_start(t[:], wred_in[k0:k1, :])
                wredsb.append(t)
            bredsb = []
            for oi, (o0, o1) in enumerate(kch(f_red_out)):
                t = res.tile([o1 - o0, 1], f32, tag=f"bred_{oi}")
                nc.sync.dma_start(t[:], bred_in[o0:o1, :])
                bredsb.append(t)
            idxsb = []
            for w in range(nw):
                t = res.tile([P, nt * cw[w] * 8], i16, tag=f"idx{w}")
                nc.sync.dma_start(t[:], idx_in[w][:])
                idxsb.append(t)
            dstlocsb = res.tile([P, nt * ctot], f32, tag="dstloc")
            nc.sync.dma_start(dstlocsb[:], dstloc_in[:])
            invdegsb = res.tile([P, nt], f32, tag="invdeg")
            nc.sync.dma_start(invdegsb[:], invdeg_in[:])
            ident = res.tile([P, P], f32, tag="ident")
            make_identity(nc, ident[:])
            iota_i = res.tile([P, P], i32, tag="iota_i")
            nc.gpsimd.iota(iota_i[:], pattern=[[1, P]], base=0, channel_multiplier=0)
            iotasb = res.tile([P, P], f32, tag="iota_f")
            nc.vector.tensor_copy(iotasb[:], iota_i[:])
            maxacc = res.tile([P, f_red_out // P], f32, tag="maxacc")
            nc.vector.memset(maxacc[:], -3.0e38)

            # working pools shared by the ZERON layers
            xt_pool = ctx.enter_context(tc.tile_pool(name="xt", bufs=3))
            work = ctx.enter_context(tc.tile_pool(name="work", bufs=3))
            spool = ctx.enter_context(tc.tile_pool(name="sel", bufs=4))
            stage = ctx.enter_context(tc.tile_pool(name="stage", bufs=2))
            zpool = ctx.enter_context(tc.tile_pool(name="z", bufs=3))

            def elu_store(z, fo, dst_ap):
                """z: SBUF [fo,P] pre-activation; writes elu(z) to dst_ap."""
                t1 = zpool.tile([fo, P], f32, tag="elu_min")
                nc.vector.tensor_scalar_min(t1[:], z[:], 0.0)
                e = zpool.tile([fo, P], f32, tag="elu_exp")
                nc.scalar.activation(e[:], t1[:], Act.Exp)
                t3 = zpool.tile([fo, P], f32, tag="elu_max")
                nc.vector.tensor_scalar(t3[:], z[:], 0.0, -1.0, Alu.max, Alu.add)
                xo = zpool.tile([fo, P], f32, tag="elu_out")
                nc.vector.tensor_add(xo[:], e[:], t3[:])
                nc.sync.dma_start(dst_ap, xo[:])

            gcap = int(os.environ.get("BASS_GATHER_CAP", "1024"))

            def emit_gather(st, w, rw_, col0, ncols):
                """Gather ncols chunks (128 idx each) starting at global chunk
                col0 of window w into st, split into <=gcap-idx instructions."""
                ccap = max(1, gcap // P)
                for c in range(0, ncols, ccap):
                    cn = min(ccap, ncols - c)
                    nids = cn * P
                    i0 = (col0 + c) * 8
                    nc.gpsimd.dma_gather(
                        out_ap=st[:, c:c + cn, :],
                        in_ap=tbl_full[rw_][w * wrows:(w + 1) * wrows, :],
                        idxs_ap=idxsb[w][:, i0:i0 + nids // 16],
                        num_idxs=nids, num_idxs_reg=nids, elem_size=rw_)

            def build_sel(col):
                S = spool.tile([P, P], f32, tag="S")
                nc.vector.tensor_tensor(
                    S[:], dstlocsb[:, col:col + 1].to_broadcast([P, P]),
                    iotasb[:], op=Alu.is_equal)
                return S

            with contextlib.ExitStack() as zctx:
                psA = zctx.enter_context(tc.tile_pool(name="psA", bufs=2, space="PSUM"))
                psB = zctx.enter_context(tc.tile_pool(name="psB", bufs=2, space="PSUM"))
                psC = zctx.enter_context(tc.tile_pool(name="psC", bufs=2, space="PSUM"))
                psD = zctx.enter_context(tc.tile_pool(name="psD", bufs=2, space="PSUM"))

                for l in range(nlay):
                    fi, fo = dims[l]
                    s, rw = sides[l], rws[l]
                    ks = kch(fi)
                    pch = kch(fo - s)          # passthrough row chunks
                    # ---- pass A: message table for this layer ----
                    for t in range(nt):
                        xts = []
                        for (k0, k1) in ks:
                            xt = xt_pool.tile([k1 - k0, P], f32, tag="xtA")
                            nc.sync.dma_start(
                                xt[:], xt_dram[l][k0:k1, t * P:(t + 1) * P])
                            xts.append(xt)
                        sps = psA.tile([s, P], f32, tag="side")
                        for ki, (k0, k1) in enumerate(ks):
                            nc.tensor.matmul(sps[:], lhsT=wsb[l][ki][:, 0:s],
                                             rhs=xts[ki][:],
                                             start=(ki == 0), stop=(ki == len(ks) - 1))
                        ssb = work.tile([s, P], f32, tag="side_sb")
                        nc.vector.tensor_copy(ssb[:], sps[:])
                        trp = psB.tile([P, s], f32, tag="tr")
                        nc.tensor.transpose(trp[:], ssb[:], ident[0:s, 0:s])
                        tsb = work.tile([P, rw], f32, tag="tbl_sb")
                        nc.vector.tensor_scalar_mul(tsb[:, 0:s], trp[:],
                                                    invdegsb[:, t:t + 1])
                        if rw > s:
                            nc.vector.memset(tsb[:, s:rw], 0.0)
                        nc.sync.dma_start(tbl_shard[rw][t * P:(t + 1) * P, :], tsb[:])
                    nc.gpsimd.collective_compute(
                        "AllGather", Alu.bypass, replica_groups=rg,
                        ins=[tbl_shard[rw][:]], outs=[tbl_full[rw][:]])
                    # ---- pass C: aggregate + combine ----
                    tgl = _pick_tg(nt, tg if rw <= 64 else 2)
                    for g in range(nt // tgl):
                        sts = []
                        for w in range(nw):
                            st = stage.tile([P, tgl * cw[w], rw], f32, tag=f"st{w}")
                            emit_gather(st, w, rw, g * tgl * cw[w], tgl * cw[w])
                            sts.append(st)
                        for ti in range(tgl):
                            t = g * tgl + ti
                            agg = psC.tile([rw, P], f32, tag="agg")
                            first = True
                            for w in range(nw):
                                for k in range(cw[w]):
                                    S = build_sel(t * ctot + woff[w] + k)
                                    nc.tensor.matmul(
                                        agg[:], lhsT=sts[w][:, ti * cw[w] + k, :],
                                        rhs=S[:], start=first,
                                        stop=(w == nw - 1 and k == cw[w] - 1))
                                    first = False
                            z = zpool.tile([s, P], f32, tag="z")
                            nc.vector.tensor_scalar_add(z[:], agg[0:s, :],
                                                        bsb_agg[l][:])
                            elu_store(z, s, xt_dram[l + 1][0:s, t * P:(t + 1) * P])
                            xts = []
                            for (k0, k1) in ks:
                                xt = xt_pool.tile([k1 - k0, P], f32, tag="xtC")
                                nc.sync.dma_start(
                                    xt[:], xt_dram[l][k0:k1, t * P:(t + 1) * P])
                                xts.append(xt)
                            for pi, (p0, p1) in enumerate(pch):
                                pps = psD.tile([p1 - p0, P], f32, tag="pass")
                                for ki, (k0, k1) in enumerate(ks):
                                    nc.tensor.matmul(
                                        pps[:], lhsT=wsb[l][ki][:, s + p0:s + p1],
                                        rhs=xts[ki][:], start=(ki == 0),
                                        stop=(ki == len(ks) - 1))
                                z = zpool.tile([p1 - p0, P], f32, tag="z")
                                nc.vector.tensor_scalar_add(
                                    z[:], pps[:], bsb_pass[l][pi][:])
                                elu_store(z, p1 - p0,
                                          xt_dram[l + 1][s + p0:s + p1,
                                                         t * P:(t + 1) * P])

            # ---------------- reduce layer (GCNMax) ----------------
            mch = kch(rw_red)          # M chunks of the gathered row
            kred = kch(f_red_in)       # K chunks for W_red
            och = kch(f_red_out)       # output channel chunks
            with contextlib.ExitStack() as rctx:
                psA2 = [rctx.enter_context(
                    tc.tile_pool(name=f"psR{j}", bufs=1, space="PSUM"))
                    for j in range(len(mch))]
                psO = rctx.enter_context(tc.tile_pool(name="psO", bufs=2, space="PSUM"))
                psT2 = rctx.enter_context(tc.tile_pool(name="psT2", bufs=2, space="PSUM"))
                # pass A: table = raw x rows
                for t in range(nt):
                    tsb = work.tile([P, rw_red], f32, tag="tblR_sb")
                    for (r0, r1) in kred:
                        xt = xt_pool.tile([r1 - r0, P], f32, tag="xtR")
                        nc.sync.dma_start(xt[:], xt_dram[nlay][r0:r1, t * P:(t + 1) * P])
                        trp = psT2.tile([P, r1 - r0], f32, tag="trR")
                        nc.tensor.transpose(trp[:], xt[:], ident[0:r1 - r0, 0:r1 - r0])
                        nc.vector.tensor_copy(tsb[:, r0:r1], trp[:])
                    if rw_red > f_red_in:
                        nc.vector.memset(tsb[:, f_red_in:rw_red], 0.0)
                    nc.sync.dma_start(tbl_shard[rw_red][t * P:(t + 1) * P, :], tsb[:])
                nc.gpsimd.collective_compute(
                    "AllGather", Alu.bypass, replica_groups=rg,
                    ins=[tbl_shard[rw_red][:]], outs=[tbl_full[rw_red][:]])
                # pass C
                for g in range(nt // tg_red):
                    sts = []
                    for w in range(nw):
                        st = stage.tile([P, tg_red * cw[w], rw_red], f32, tag=f"st{w}")
                        emit_gather(st, w, rw_red, g * tg_red * cw[w], tg_red * cw[w])
                        sts.append(st)
                    for ti in range(tg_red):
                        t = g * tg_red + ti
                        aps = [psA2[j].tile([m1 - m0, P], f32, tag=f"aggR{j}",
                                            name=f"aggR{j}")
                               for j, (m0, m1) in enumerate(mch)]
                        first = True
                        for w in range(nw):
                            for k in range(cw[w]):
                                S = build_sel(t * ctot + woff[w] + k)
                                for j, (m0, m1) in enumerate(mch):
                                    nc.tensor.matmul(
                                        aps[j][:],
                                        lhsT=sts[w][:, ti * cw[w] + k, m0:m1],
                                        rhs=S[:], start=first,
                                        stop=(w == nw - 1 and k == cw[w] - 1))
                                first = False
                        asb = []
                        for j, (m0, m1) in enumerate(mch):
                            hi = min(m1, f_red_in) - m0
                            a = work.tile([hi, P], f32, tag=f"aggRs{j}")
                            nc.vector.tensor_copy(a[:], aps[j][0:hi, :])
                            asb.append(a)
                        valid = P
                        if t == nt - 1 and plan["nsh_real"] % P:
                            valid = plan["nsh_real"] % P
                        for oi, (o0, o1) in enumerate(och):
                            ops = psO.tile([o1 - o0, P], f32, tag="outR")
                            for j, (k0, k1) in enumerate(kred):
                                nc.tensor.matmul(ops[:],
                                                 lhsT=wredsb[j][:, o0:o1],
                                                 rhs=asb[j][:], start=(j == 0),
                                                 stop=(j == len(kred) - 1))
                            z = zpool.tile([o1 - o0, P], f32, tag="zR")
                            nc.vector.tensor_scalar_add(z[:], ops[:], bredsb[oi][:])
                            t1 = zpool.tile([o1 - o0, P], f32, tag="zR1")
                            nc.vector.tensor_scalar_min(t1[:], z[:], 0.0)
                            e = zpool.tile([o1 - o0, P], f32, tag="zR2")
                            nc.scalar.activation(e[:], t1[:], Act.Exp)
                            t3 = zpool.tile([o1 - o0, P], f32, tag="zR3")
                            nc.vector.tensor_scalar(t3[:], z[:], 0.0, -1.0,
                                                    Alu.max, Alu.add)
                            o = zpool.tile([o1 - o0, P], f32, tag="zR4")
                            nc.vector.tensor_add(o[:], e[:], t3[:])
                            red = zpool.tile([o1 - o0, 1], f32, tag="zR5")
                            nc.vector.tensor_reduce(red[:], o[:, 0:valid],
                                                    axis=mybir.AxisListType.X,
                                                    op=Alu.max)
                            nc.vector.tensor_max(maxacc[:, oi:oi + 1],
                                                 maxacc[:, oi:oi + 1], red[:])
                nc.sync.dma_start(maxsh[:], maxacc[:])
                nc.gpsimd.collective_compute(
                    "AllReduce", Alu.max, replica_groups=rg,
                    ins=[maxsh[:]], outs=[maxfull[:]])
                nc.sync.dma_start(out_ext[:], maxfull[:])

    nc.compile()
    return nc


# ---------------------------------------------------------------------------
# Entry point
# ---------------------------------------------------------------------------

def _make_inmaps(plan, positions, Ws, bs, dims, red_dim):
    nsh, nsh_real = plan["nsh"], plan["nsh_real"]
    in_maps = []
    for c in range(NCORES):
        m = {}
        sh = np.zeros((nsh, positions.shape[1]), dtype=np.float32)
        lo, hi = c * nsh_real, min((c + 1) * nsh_real, positions.shape[0])
        sh[:hi - lo] = positions[lo:hi]
        m["posT"] = np.ascontiguousarray(sh.T)
        for l in range(len(dims)):
            m[f"W{l}"] = np.ascontiguousarray(Ws[l], dtype=np.float32)
            m[f"b{l}"] = np.ascontiguousarray(bs[l], dtype=np.float32).reshape(-1, 1)
        m["Wred"] = np.ascontiguousarray(Ws[len(dims)], dtype=np.float32)
        m["bred"] = np.ascontiguousarray(bs[len(dims)], dtype=np.float32).reshape(-1, 1)
        for w in range(len(plan["cw"])):
            m[f"idx{w}"] = plan["idx"][c][w]
        m["dstloc"] = plan["dstloc"][c]
        m["invdeg"] = plan["invdeg"][c]
        in_maps.append(m)
    return in_maps


_CACHE = {}
LAST_RESULTS = None


def _pick_tg(nt, target):
    for d in range(min(target, nt), 0, -1):
        if nt % d == 0:
            return d
    return 1


def kernel(positions, edge_src, edge_dst, Ws, bs):
    global LAST_RESULTS
    positions = np.asarray(positions, dtype=np.float32)
    n = positions.shape[0]
    nsh_real = (n + NCORES - 1) // NCORES
    plan = _graph_plan(edge_src, edge_dst, n, NCORES, nsh_real, 4)

    key = ("prog", tuple(plan["cw"]), plan["nsh"])
    if key not in _CACHE:
        nt = plan["nt"]
        _CACHE[key] = _build_program(plan, ZERON_DIMS, RED_DIM,
                                     tg=_pick_tg(nt, 7), tg_red=1)
    nc = _CACHE[key]

    in_maps = _make_inmaps(plan, positions, Ws, bs, ZERON_DIMS, RED_DIM)
    from concourse.bass_utils import run_bass_kernel_spmd
    kw = {}
    if os.environ.get("KERNEL_TRACE"):
        kw = dict(trace=True, tmpdir=os.environ.get("KERNEL_TRACE_DIR") or None)
    try:
        res = run_bass_kernel_spmd(nc, in_maps, core_ids=list(range(NCORES)), **kw)
    except ModuleNotFoundError:
        res = run_bass_kernel_spmd(nc, in_maps, core_ids=list(range(NCORES)))
    LAST_RESULTS = res
    out = np.asarray(res.results[0]["out"])          # [128, 4]
    return np.ascontiguousarray(out.T).reshape(-1)   # [512]
